# revision 1
# baseline (speedup 1.0000x reference)
"""CrossTransformer kernel for Trainium2, data-parallel over batch across 8 cores.

Math per batch b (B=32, N=25, C=512, H=W=14, DK=DV=128):
  qq = Wqk @ Q    [128, 196]      qv = Wv @ Q     [128, 196]
  K  = Wqk @ S    [128, 4900]     V  = Wv @ S     [128, 4900]
  simT[nij, hw] = K^T @ qq        (computed directly in transposed layout)
  E = exp(simT)                   (no max subtraction; |sim| <~ 60 is safe in fp32)
  ctx_raw[hw, v+1] = sum_nij E[nij, hw]^T @ [V^T | 1]   (ones column -> softmax denom)
  ctx = ctx_raw[:, :128] / ctx_raw[:, 128:129]
  partial += sum((qv^T - ctx)^2)
Output per core: scalar partial sum over its 4 batches; host sums and divides by H*W.
"""

import os
import sys

sys.path.insert(0, "/opt/trn_rl_repo")

import numpy as np

import concourse.bass as bass
import concourse.bacc as bacc
import concourse.mybir as mybir
import concourse.tile as tile
from concourse.bass_utils import run_bass_kernel_spmd
from concourse.masks import make_identity

F32 = mybir.dt.float32
F32R = mybir.dt.float32r
BF16 = mybir.dt.bfloat16

B_PER_CORE = 4
N_SUP = 25
C = 512
HW = 196
NIJ = N_SUP * HW  # 4900
DK = 128
NCH = (NIJ + 127) // 128  # 39 chunks of <=128 along nij
FT = 490                   # nij tile width for projections (fits one PSUM bank)
NT = NIJ // FT             # 10
CCH = C // 128             # 4 c-chunks


def _spans(start, end):
    """Split nij range [start,end) into DMA pieces aligned to n boundaries.
    Returns (n0, nn, ij0, L, dstoff) tuples; full-n middle merged into one."""
    res = []
    cur = start
    if cur % HW != 0:
        n = cur // HW
        ij0 = cur % HW
        L = min(HW - ij0, end - cur)
        res.append((n, 1, ij0, L, cur - start))
        cur += L
    nfull = (end - cur) // HW
    if nfull > 0:
        res.append((cur // HW, nfull, 0, HW, cur - start))
        cur += nfull * HW
    if cur < end:
        res.append((cur // HW, 1, 0, end - cur, cur - start))
    return res


def build_bass():
    nc = bacc.Bacc(
        "TRN2", target_bir_lowering=False, debug=False, enable_asserts=False
    )
    q_d = nc.dram_tensor("q", [B_PER_CORE, C, HW], F32, kind="ExternalInput").ap()
    s_d = nc.dram_tensor(
        "s", [B_PER_CORE, N_SUP, C, HW], F32, kind="ExternalInput"
    ).ap()
    wqk_d = nc.dram_tensor("wqk", [DK, C], F32, kind="ExternalInput").ap()
    wv_d = nc.dram_tensor("wv", [DK, C], F32, kind="ExternalInput").ap()
    out_d = nc.dram_tensor("out", [1, 1], F32, kind="ExternalOutput").ap()

    with tile.TileContext(nc) as tc:
        with (
            tc.tile_pool(name="const", bufs=1) as const,
            tc.tile_pool(name="spool", bufs=16) as spool,
            tc.tile_pool(name="kvbf", bufs=2) as kvbf,
            tc.tile_pool(name="vt1p", bufs=2 * NCH + 4) as vt1p,
            tc.tile_pool(name="etp", bufs=NCH + 5) as etp,
            tc.tile_pool(name="small", bufs=4) as small,
            tc.tile_pool(name="ps_proj", bufs=3, space="PSUM") as ps_proj,
            tc.tile_pool(name="ps_sim", bufs=2, space="PSUM") as ps_sim,
            tc.tile_pool(name="ps_vt", bufs=2, space="PSUM") as ps_vt,
            tc.tile_pool(name="ps_ctx", bufs=1, space="PSUM") as ps_ctx,
        ):
            # ---- constants / weights ----
            id_f32 = const.tile([128, 128], F32, tag="id_f32")
            make_identity(nc, id_f32)
            id_bf = const.tile([128, 128], BF16, tag="id_bf")
            make_identity(nc, id_bf)

            wqk_sb = const.tile([128, C], F32, tag="wqk_sb")
            nc.sync.dma_start(out=wqk_sb, in_=wqk_d)
            wv_sb = const.tile([128, C], F32, tag="wv_sb")
            nc.sync.dma_start(out=wv_sb, in_=wv_d)

            wqkT = []
            wvT = []
            for cc in range(CCH):
                for (src, dstl, nm) in ((wqk_sb, wqkT, "qk"), (wv_sb, wvT, "v")):
                    pt = ps_vt.tile([128, 128], F32, tag="ps_vt")
                    nc.tensor.transpose(pt, src[:, cc * 128 : (cc + 1) * 128], id_f32)
                    wt = const.tile([128, 128], F32R, tag=f"w{nm}T{cc}")
                    nc.vector.tensor_copy(wt, pt)
                    dstl.append(wt)

            # ---- query load + projections (all 4 batches at once) ----
            qsb = []
            for cc in range(CCH):
                qt = const.tile([128, B_PER_CORE * HW], F32R, tag=f"qsb{cc}")
                src = q_d[:, cc * 128 : (cc + 1) * 128, :].rearrange(
                    "b c ij -> c b ij"
                ).bitcast(F32R)
                nc.sync.dma_start(
                    out=qt.rearrange("p (b ij) -> p b ij", b=B_PER_CORE), in_=src
                )
                qsb.append(qt)

            qq_bf = const.tile([128, B_PER_CORE * HW], BF16, tag="qq_bf")
            qv_sb = const.tile([128, B_PER_CORE * HW], F32, tag="qv_sb")
            for wT, dst in ((wqkT, qq_bf), (wvT, qv_sb)):
                for half in range(2):
                    hw0 = half * 392
                    pq = ps_proj.tile([128, FT], F32, tag="ps_proj")
                    for cc in range(CCH):
                        nc.tensor.matmul(
                            pq[:, :392],
                            lhsT=wT[cc],
                            rhs=qsb[cc][:, hw0 : hw0 + 392],
                            start=(cc == 0),
                            stop=(cc == CCH - 1),
                        )
                    nc.vector.tensor_copy(dst[:, hw0 : hw0 + 392], pq[:, :392])

            # qv^T per (b, hw-chunk): [hw<=128, 128] fp32 — matches ctx layout
            qvT = {}
            for b in range(B_PER_CORE):
                for h in range(2):
                    hww = 128 if h == 0 else HW - 128
                    pt = ps_vt.tile([128, 128], F32, tag="ps_vt")
                    nc.tensor.transpose(
                        pt[:hww, :],
                        qv_sb[:, b * HW + h * 128 : b * HW + h * 128 + hww],
                        id_f32,
                    )
                    qt = const.tile([128, 128], F32, tag=f"qvT{b}_{h}")
                    nc.vector.tensor_copy(qt[:hww, :], pt[:hww, :])
                    qvT[(b, h)] = qt

            partials = const.tile([128, 2 * B_PER_CORE], F32, tag="partials")
            nc.vector.memset(partials, 0.0)

            # ---- per-batch main pipeline ----
            import os as _os
            KPHASES = int(_os.environ.get("KPHASES", "4"))
            for b in range(B_PER_CORE):
                k_bf = kvbf.tile([128, NIJ], BF16, tag="k_bf")
                v_bf = kvbf.tile([128, NIJ], BF16, tag="v_bf")

                # projections: stream S in FT-wide nij tiles
                for t in range(NT):
                    st = []
                    for cc in range(CCH):
                        s_t = spool.tile([128, FT], F32R, tag="s_t")
                        for (n0, nn, ij0, L, off) in _spans(t * FT, (t + 1) * FT):
                            src = s_d[
                                b, n0 : n0 + nn, cc * 128 : (cc + 1) * 128,
                                ij0 : ij0 + L,
                            ].rearrange("n c ij -> c n ij").bitcast(F32R)
                            nc.sync.dma_start(
                                out=s_t[:, off : off + nn * L].rearrange(
                                    "p (n ij) -> p n ij", n=nn
                                ),
                                in_=src,
                            )
                        st.append(s_t)
                    pk = ps_proj.tile([128, FT], F32, tag="ps_proj")
                    for cc in range(CCH):
                        nc.tensor.matmul(
                            pk,
                            lhsT=wqkT[cc],
                            rhs=st[cc],
                            start=(cc == 0),
                            stop=(cc == CCH - 1),
                        )
                    nc.vector.tensor_copy(k_bf[:, t * FT : (t + 1) * FT], pk)
                    pv = ps_proj.tile([128, FT], F32, tag="ps_proj")
                    for cc in range(CCH):
                        nc.tensor.matmul(
                            pv,
                            lhsT=wvT[cc],
                            rhs=st[cc],
                            start=(cc == 0),
                            stop=(cc == CCH - 1),
                        )
                    nc.scalar.copy(v_bf[:, t * FT : (t + 1) * FT], pv)

                # V^T chunks (+ ones column) via PE transpose
                vt1 = []
                if KPHASES < 2:
                    continue
                for j in range(NCH):
                    cw = min(128, NIJ - j * 128)
                    vt = vt1p.tile([128, 132], BF16, tag="vt1")
                    if cw < 128:
                        nc.vector.memset(vt, 0.0)
                    pt = ps_vt.tile([128, 128], BF16, tag="ps_vt")
                    nc.tensor.transpose(
                        pt[:cw, :], v_bf[:, j * 128 : j * 128 + cw], id_bf
                    )
                    nc.vector.tensor_copy(vt[:cw, 0:128], pt[:cw, :])
                    nc.vector.memset(vt[:, 128:132], 1.0)
                    vt1.append(vt)

                # simT = K^T @ qq (bf16), exp -> E^T chunks
                et = []
                if KPHASES < 3:
                    continue
                for j in range(NCH):
                    cw = min(128, NIJ - j * 128)
                    ps = ps_sim.tile([128, HW], F32, tag="ps_sim")
                    nc.tensor.matmul(
                        ps[:cw, :],
                        lhsT=k_bf[:, j * 128 : j * 128 + cw],
                        rhs=qq_bf[:, b * HW : (b + 1) * HW],
                        start=True,
                        stop=True,
                    )
                    e = etp.tile([128, HW], BF16, tag="et")
                    if cw < 128:
                        nc.vector.memset(e, 0.0)
                    nc.scalar.activation(
                        out=e[:cw, :],
                        in_=ps[:cw, :],
                        func=mybir.ActivationFunctionType.Exp,
                    )
                    et.append(e)

                # PV: ctx_raw[hw, 129] accumulated over 39 nij chunks
                if KPHASES < 4:
                    continue
                for h in range(2):
                    hww = 128 if h == 0 else HW - 128
                    pc = ps_ctx.tile([128, 132], F32, tag="ps_ctx")
                    for j in range(NCH):
                        nc.tensor.matmul(
                            pc[:hww, 0:132],
                            lhsT=et[j][:, h * 128 : h * 128 + hww],
                            rhs=vt1[j][:, 0:132],
                            start=(j == 0),
                            stop=(j == NCH - 1),
                        )
                    r = small.tile([128, 1], F32, tag="recip")
                    nc.vector.reciprocal(r[:hww], pc[:hww, 128:129])
                    ctx = small.tile([128, 128], F32, tag="ctx")
                    nc.vector.tensor_scalar_mul(
                        ctx[:hww, :], pc[:hww, 0:128], r[:hww]
                    )
                    d = small.tile([128, 128], F32, tag="diff")
                    nc.vector.tensor_sub(
                        d[:hww, :], qvT[(b, h)][:hww, :], ctx[:hww, :]
                    )
                    d2 = small.tile([128, 128], F32, tag="d2")
                    nc.vector.tensor_mul(d2[:hww, :], d[:hww, :], d[:hww, :])
                    nc.vector.reduce_sum(
                        partials[:hww, 2 * b + h : 2 * b + h + 1],
                        d2[:hww, :],
                        axis=mybir.AxisListType.X,
                    )

            # ---- final reduction to scalar ----
            tot = small.tile([128, 1], F32, tag="tot")
            nc.vector.reduce_sum(tot, partials, axis=mybir.AxisListType.X)
            ones = small.tile([128, 1], F32, tag="ones")
            nc.vector.memset(ones, 1.0)
            pf = ps_vt.tile([128, 128], F32, tag="ps_vt")
            nc.tensor.matmul(pf[0:1, 0:1], lhsT=tot, rhs=ones, start=True, stop=True)
            ob = small.tile([1, 1], F32, tag="ob")
            nc.vector.tensor_copy(ob, pf[0:1, 0:1])
            nc.sync.dma_start(out=out_d, in_=ob)

    nc.compile()
    return nc


_NC = None


def kernel(query_repr, supports_repr, W_qk, W_v):
    global _NC
    q = np.ascontiguousarray(np.asarray(query_repr, dtype=np.float32)).reshape(
        32, C, HW
    )
    s = np.ascontiguousarray(np.asarray(supports_repr, dtype=np.float32)).reshape(
        32, N_SUP, C, HW
    )
    wqk = np.ascontiguousarray(np.asarray(W_qk, dtype=np.float32))
    wv = np.ascontiguousarray(np.asarray(W_v, dtype=np.float32))

    if _NC is None:
        _NC = build_bass()

    in_maps = []
    for core in range(8):
        b0 = core * B_PER_CORE
        in_maps.append(
            {
                "q": np.ascontiguousarray(q[b0 : b0 + B_PER_CORE]),
                "s": np.ascontiguousarray(s[b0 : b0 + B_PER_CORE]),
                "wqk": wqk,
                "wv": wv,
            }
        )
    res = run_bass_kernel_spmd(
        _NC, in_maps, core_ids=list(range(8)),
        trace=bool(int(os.environ.get("KTRACE", "0"))),
    )
    total = sum(float(r["out"][0, 0]) for r in res.results) / float(HW)
    kernel._last_results = res
    return np.asarray(total, dtype=np.float32)



# revision 58
# speedup vs baseline: 1.6246x; 1.6246x over previous
"""CrossTransformer kernel for Trainium2, data-parallel over batch across 8 cores.

Math per batch b (B=32, N=25, C=512, H=W=14, DK=DV=128):
  qq = Wqk @ Q    [128, 196]      qv = Wv @ Q     [128, 196]
  K  = Wqk @ S    [128, 4900]     V  = Wv @ S     [128, 4900]
  simT[nij, hw] = K^T @ qq        (computed directly in transposed layout)
  E = exp(simT)                   (no max subtraction; |sim| <~ 60 is safe in fp32)
  ctx_raw[hw, v+1] = sum_nij E[nij, hw]^T @ [V^T | 1]   (ones column -> softmax denom)
  ctx = ctx_raw[:, :128] / ctx_raw[:, 128:129]
  partial += sum((qv^T - ctx)^2)
Output per core: scalar partial sum over its 4 batches; host sums and divides by H*W.
"""

import os
import sys

sys.path.insert(0, "/opt/trn_rl_repo")

import numpy as np

import concourse.bass as bass
import concourse.bacc as bacc
import concourse.mybir as mybir
import concourse.tile as tile
from concourse.bass_utils import run_bass_kernel_spmd
from concourse.masks import make_identity

F32 = mybir.dt.float32
F32R = mybir.dt.float32r
BF16 = mybir.dt.bfloat16

B_PER_CORE = 4
N_SUP = 25
C = 512
HW = 196
NIJ = N_SUP * HW  # 4900
DK = 128
NCH = (NIJ + 127) // 128  # 39 chunks of <=128 along nij
FT = 490                   # nij tile width for projections (fits one PSUM bank)
NT = NIJ // FT             # 10
CCH = C // 128             # 4 c-chunks
GN = 5                     # support images per DMA group (n-aligned loads)
GW = GN * HW               # 980 nij per group
NG = N_SUP // GN           # 5 groups
# per-batch n-group schedules: batch 0 front-loads a small group so the
# pipeline fills fast; the last batch ends with a tiny group so the drain
# dependency chain after the final DMA is short.
GROUPS = [
    [2, 5, 6, 6, 6],
    [5, 5, 5, 5, 5],
    [5, 5, 5, 5, 5],
    [6, 6, 6, 5, 2],
]
GWMAX = 6 * HW


def _spans(start, end):
    """Split nij range [start,end) into DMA pieces aligned to n boundaries.
    Returns (n0, nn, ij0, L, dstoff) tuples; full-n middle merged into one."""
    res = []
    cur = start
    if cur % HW != 0:
        n = cur // HW
        ij0 = cur % HW
        L = min(HW - ij0, end - cur)
        res.append((n, 1, ij0, L, cur - start))
        cur += L
    nfull = (end - cur) // HW
    if nfull > 0:
        res.append((cur // HW, nfull, 0, HW, cur - start))
        cur += nfull * HW
    if cur < end:
        res.append((cur // HW, 1, 0, end - cur, cur - start))
    return res


def _pv_accum(nc, pc, e, vt, j):
    """Accumulate PV for chunk j into the two hw-half PSUM accumulators."""
    for h in range(2):
        hww = 128 if h == 0 else HW - 128
        nc.tensor.matmul(
            pc[h][:hww, 0:132],
            lhsT=e[:, h * 128 : h * 128 + hww],
            rhs=vt,
            start=(j == 0),
            stop=(j == NCH - 1),
        )


def build_bass():
    nc = bacc.Bacc(
        "TRN2", target_bir_lowering=False, debug=False, enable_asserts=False
    )
    q_d = nc.dram_tensor("q", [B_PER_CORE, C, HW], F32, kind="ExternalInput").ap()
    s_d = nc.dram_tensor(
        "s", [B_PER_CORE, N_SUP, C, HW], F32, kind="ExternalInput"
    ).ap()
    wqk_d = nc.dram_tensor("wqk", [DK, C], F32, kind="ExternalInput").ap()
    wv_d = nc.dram_tensor("wv", [DK, C], F32, kind="ExternalInput").ap()
    out_d = nc.dram_tensor(
        "out", [B_PER_CORE, 128, 2], F32, kind="ExternalOutput"
    ).ap()

    with tile.TileContext(nc) as tc:
        with (
            tc.tile_pool(name="const", bufs=1) as const,
            tc.tile_pool(name="spool", bufs=16) as spool,
            tc.tile_pool(name="kvbf", bufs=4) as kvbf,
            tc.tile_pool(name="etp", bufs=8) as etp,
            tc.tile_pool(name="small", bufs=4) as small,
            tc.tile_pool(name="ps_proj", bufs=3, space="PSUM") as ps_proj,
            tc.tile_pool(name="ps_sim", bufs=2, space="PSUM") as ps_sim,
            tc.tile_pool(name="ps_vt", bufs=1, space="PSUM") as ps_vt,
            tc.tile_pool(name="ps_ctx", bufs=2, space="PSUM") as ps_ctx,
        ):
            # ---- constants / weights ----
            id_f32 = const.tile([128, 128], F32, tag="id_f32")
            make_identity(nc, id_f32)
            id_bf = const.tile([128, 128], BF16, tag="id_bf")
            make_identity(nc, id_bf)

            wqk_sb = const.tile([128, C], F32, tag="wqk_sb")
            nc.sync.dma_start(out=wqk_sb, in_=wqk_d)
            wv_sb = const.tile([128, C], F32, tag="wv_sb")
            nc.sync.dma_start(out=wv_sb, in_=wv_d)

            wqkT = []
            wvT = []
            for cc in range(CCH):
                for (src, dstl, nm) in ((wqk_sb, wqkT, "qk"), (wv_sb, wvT, "v")):
                    pt = ps_proj.tile([128, FT], F32, tag="ps_proj")
                    nc.tensor.transpose(
                        pt[:, 0:128], src[:, cc * 128 : (cc + 1) * 128], id_f32
                    )
                    wt = const.tile([128, 128], F32R, tag=f"w{nm}T{cc}")
                    nc.vector.tensor_copy(wt, pt[:, 0:128])
                    dstl.append(wt)

            # ---- query load + projections (all 4 batches at once) ----
            # q loads issued from the Act engine queue so the SP queue starts
            # streaming S immediately after the weights.
            qsb = []
            for cc in range(CCH):
                qt = const.tile([128, B_PER_CORE * HW], F32R, tag=f"qsb{cc}")
                src = q_d[:, cc * 128 : (cc + 1) * 128, :].rearrange(
                    "b c ij -> c b ij"
                ).bitcast(F32R)
                nc.sync.dma_start(
                    out=qt.rearrange("p (b ij) -> p b ij", b=B_PER_CORE), in_=src
                )
                qsb.append(qt)

            qq_bf = const.tile([128, B_PER_CORE * HW], BF16, tag="qq_bf")
            qv_sb = const.tile([128, B_PER_CORE * HW], F32, tag="qv_sb")
            for wT, dst in ((wqkT, qq_bf), (wvT, qv_sb)):
                for half in range(2):
                    hw0 = half * 392
                    pq = ps_proj.tile([128, FT], F32, tag="ps_proj")
                    for cc in range(CCH):
                        nc.tensor.matmul(
                            pq[:, :392],
                            lhsT=wT[cc],
                            rhs=qsb[cc][:, hw0 : hw0 + 392],
                            start=(cc == 0),
                            stop=(cc == CCH - 1),
                        )
                    nc.vector.tensor_copy(dst[:, hw0 : hw0 + 392], pq[:, :392])

            # qv^T per (b, hw-chunk): [hw<=128, 128] fp32 — matches ctx layout
            qvT = {}
            for b in range(B_PER_CORE):
                for h in range(2):
                    hww = 128 if h == 0 else HW - 128
                    pt = ps_proj.tile([128, FT], F32, tag="ps_proj")
                    nc.tensor.transpose(
                        pt[:hww, 0:128],
                        qv_sb[:, b * HW + h * 128 : b * HW + h * 128 + hww],
                        id_f32,
                    )
                    qt = const.tile([128, 128], F32, tag=f"qvT{b}_{h}")
                    nc.vector.tensor_copy(qt[:hww, :], pt[:hww, 0:128])
                    qvT[(b, h)] = qt

            partials = const.tile([128, 2 * B_PER_CORE], F32, tag="partials")
            nc.vector.memset(partials, 0.0)

            # persistent rotating V^T pair tiles [V0|ones|V1|ones]; ones
            # columns written once, per-pair fused copies only touch the V
            # sub-blocks (WAR deps handled by tc).
            NVTP = 4
            vtpbufs = []
            for i in range(NVTP):
                vtp = const.tile([128, 264], BF16, tag=f"vtpb{i}")
                nc.vector.memset(vtp, 1.0)
                vtpbufs.append(vtp)

            # ---- per-batch main pipeline ----
            for b in range(B_PER_CORE):
                k_bf = kvbf.tile([128, NIJ], BF16, tag="k_bf")
                v_bf = kvbf.tile([128, NIJ], BF16, tag="v_bf")
                # ctx_raw[hw, dv+ones] accumulators: one PSUM bank per
                # hw-half (a bank holds at most one pending matmul group),
                # held open across the batch so PV runs eagerly per group.
                pc0 = ps_ctx.tile([128, 132], F32, tag="ps_ctx")
                pc1 = ps_ctx.tile([128, 132], F32, tag="ps_ctx")
                pc = [pc0, pc1]
                pair = None
                pair_j0 = -1
                pt2 = None

                # stream S in n-aligned groups, one DMA per (group, cc) —
                # few HWDGE issues, full 784B descriptors.
                jdone = 0
                n0 = 0
                for gn in GROUPS[b]:
                    gw = gn * HW
                    goff = n0 * HW
                    st = []
                    for cc in range(CCH):
                        s_t = spool.tile([128, GWMAX], F32R, tag="s_t")
                        src = s_d[
                            b, n0 : n0 + gn,
                            cc * 128 : (cc + 1) * 128, :,
                        ].rearrange("n c ij -> c n ij").bitcast(F32R)
                        nc.sync.dma_start(
                            out=s_t[:, 0:gw].rearrange(
                                "p (n ij) -> p n ij", n=gn
                            ),
                            in_=src,
                        )
                        st.append(s_t)
                    n0 += gn
                    # balanced projection pieces, all >=256 wide so fp32r
                    # matmuls run at full rate
                    npc = -(-gw // FT)
                    pws = [gw // npc + (1 if i < gw % npc else 0)
                           for i in range(npc)]
                    p0 = 0
                    for pw in pws:
                        o0 = goff + p0
                        sl0 = p0
                        p0 += pw
                        sl = slice(sl0, sl0 + pw)
                        pk = ps_proj.tile([128, FT], F32, tag="ps_proj")
                        for cc in range(CCH):
                            nc.tensor.matmul(
                                pk[:, 0:pw],
                                lhsT=wqkT[cc],
                                rhs=st[cc][:, sl],
                                start=(cc == 0),
                                stop=(cc == CCH - 1),
                            )
                        if (o0 // FT) % 2 == 0:
                            nc.vector.tensor_copy(
                                k_bf[:, o0 : o0 + pw], pk[:, 0:pw]
                            )
                        else:
                            nc.scalar.copy(k_bf[:, o0 : o0 + pw], pk[:, 0:pw])
                        pv = ps_proj.tile([128, FT], F32, tag="ps_proj")
                        for cc in range(CCH):
                            nc.tensor.matmul(
                                pv[:, 0:pw],
                                lhsT=wvT[cc],
                                rhs=st[cc][:, sl],
                                start=(cc == 0),
                                stop=(cc == CCH - 1),
                            )
                        if (o0 // FT) % 2 == 0:
                            nc.scalar.copy(v_bf[:, o0 : o0 + pw], pv[:, 0:pw])
                        else:
                            nc.vector.tensor_copy(
                                v_bf[:, o0 : o0 + pw], pv[:, 0:pw]
                            )

                    # eagerly process every fully-projected 128-chunk: simT
                    # matmuls write PAIRS of chunks into one PSUM bank as a
                    # single spanned accumulation group (start on the first
                    # write, stop on the second; the bank is lazily zeroed at
                    # start so accumulating into untouched columns is a
                    # write), one fused exp per pair; V^T transposes pair the
                    # same way in f32, one fused copy per pair; PV per chunk.
                    jmax = NCH if n0 == N_SUP else (n0 * HW) // 128
                    for j in range(jdone, jmax):
                        cw = min(128, NIJ - j * 128)
                        if pair is None:
                            pair = ps_sim.tile([128, 2 * HW], F32, tag="ps_sim")
                            pair_j0 = j
                        off = (j - pair_j0) * HW
                        nc.tensor.matmul(
                            pair[:cw, off : off + HW],
                            lhsT=k_bf[:, j * 128 : j * 128 + cw],
                            rhs=qq_bf[:, b * HW : (b + 1) * HW],
                            start=(off == 0),
                            stop=(off == HW or j == NCH - 1),
                        )

                        par = j % 2
                        if par == 0:
                            pt2 = ps_vt.tile([128, 256], BF16, tag="ps_vt2")
                        nc.tensor.matmul(
                            pt2[:cw, par * 128 : par * 128 + 128],
                            lhsT=v_bf[:, j * 128 : j * 128 + cw],
                            rhs=id_bf,
                            is_transpose=True,
                            start=(par == 0),
                            stop=(par == 1 or j == NCH - 1),
                        )
                        vtp = vtpbufs[(j // 2) % NVTP]
                        if par == 1:
                            # fused copy of the (j-1, j) transpose pair into
                            # the [V0|ones|V1|ones] tile's two V sub-blocks
                            dst = vtp.rearrange(
                                "p (two x) -> p two x", two=2
                            )[:, :, 0:128]
                            src = pt2.rearrange(
                                "p (two x) -> p two x", two=2
                            )
                            nc.vector.tensor_copy(dst, src)
                        elif j == NCH - 1:
                            nc.vector.tensor_copy(
                                vtp[:cw, 0:128], pt2[:cw, 0:128]
                            )

                        if j == pair_j0 + 1 or j == NCH - 1:
                            w = off + HW
                            e = etp.tile([128, 2 * HW], BF16, tag="et")
                            if cw < 128:
                                nc.vector.memset(e, 0.0)
                            nc.scalar.activation(
                                out=e[:cw, 0:w],
                                in_=pair[:cw, 0:w],
                                func=mybir.ActivationFunctionType.Exp,
                            )
                            for jj in range(pair_j0, j + 1):
                                vtpp = vtpbufs[(jj // 2) % NVTP]
                                _pv_accum(
                                    nc, pc, e[:, (jj - pair_j0) * HW :],
                                    vtpp[:, (jj % 2) * 132 : (jj % 2) * 132 + 132],
                                    jj,
                                )
                            pair = None
                    jdone = jmax

                # batch epilogue: softmax divide, diff vs qv^T, square,
                # reduce per-partition, then stream the [128, 2] partial
                # straight to DRAM (host does the final sum)
                for h in range(2):
                    hww = 128 if h == 0 else HW - 128
                    r = small.tile([128, 1], F32, tag="recip")
                    nc.vector.reciprocal(r[:hww], pc[h][:hww, 128:129])
                    ctx = small.tile([128, 128], F32, tag="ctx")
                    nc.vector.tensor_scalar_mul(
                        ctx[:hww, :], pc[h][:hww, 0:128], r[:hww]
                    )
                    d = small.tile([128, 128], F32, tag="diff")
                    nc.vector.tensor_sub(
                        d[:hww, :], qvT[(b, h)][:hww, :], ctx[:hww, :]
                    )
                    d2 = small.tile([128, 128], F32, tag="d2")
                    nc.vector.tensor_mul(d2[:hww, :], d[:hww, :], d[:hww, :])
                    nc.vector.reduce_sum(
                        partials[:hww, 2 * b + h : 2 * b + h + 1],
                        d2[:hww, :],
                        axis=mybir.AxisListType.X,
                    )
                nc.sync.dma_start(
                    out=out_d[b], in_=partials[:, 2 * b : 2 * b + 2]
                )

    nc.compile()
    return nc


_NC = None


def kernel(query_repr, supports_repr, W_qk, W_v):
    global _NC
    q = np.ascontiguousarray(np.asarray(query_repr, dtype=np.float32)).reshape(
        32, C, HW
    )
    s = np.ascontiguousarray(np.asarray(supports_repr, dtype=np.float32)).reshape(
        32, N_SUP, C, HW
    )
    wqk = np.ascontiguousarray(np.asarray(W_qk, dtype=np.float32))
    wv = np.ascontiguousarray(np.asarray(W_v, dtype=np.float32))

    if _NC is None:
        _NC = build_bass()

    in_maps = []
    for core in range(8):
        b0 = core * B_PER_CORE
        in_maps.append(
            {
                "q": np.ascontiguousarray(q[b0 : b0 + B_PER_CORE]),
                "s": np.ascontiguousarray(s[b0 : b0 + B_PER_CORE]),
                "wqk": wqk,
                "wv": wv,
            }
        )
    res = run_bass_kernel_spmd(
        _NC, in_maps, core_ids=list(range(8)),
        trace=bool(int(os.environ.get("KTRACE", "0"))),
    )
    total = sum(
        float(np.asarray(r["out"], dtype=np.float64).sum())
        for r in res.results
    ) / float(HW)
    kernel._last_results = res
    return np.asarray(total, dtype=np.float32)

